# revision 1
# baseline (speedup 1.0000x reference)
"""CRF forward on 8 Trainium2 cores — meet-in-the-middle bf16 chain.

answer_b = s_{len_b} = 1.T v_{len_b},  v_t = f_t * (A v_{t-1}).
Tags permuted so the dead END tag sits at partition SROW. A := exp(trans'-mu)
with row SROW := ones (sum row) and column SROW := e_SROW (self-loop weight 1).
Padded steps (f_t = e_SROW for t >= len) collapse v to s*e_SROW and hold s
exactly, so s_T is the answer for every lane. By bilinearity
s_T = r_HALF.T v_HALF with the backward functional r_{t-1} = Ab.T (f_t*r_t),
r_S = ones, Ab = A with column SROW := ones (sum monitor). Forward (512
steps) and backward (512 steps) chains run concurrently on each core ->
serial latency halves vs a single 1024-step scan. One sum-row renorm per
chain (step 256) bounds the fp range; the exact reciprocal factors are
staged out and the host adds the logs back. All tiles bf16 (PSUM fp32).
Startup DMAs fly concurrently on dedicated semaphores; later chunks
serialize on a counting semaphore (race-free thresholds)."""
import sys
import numpy as np

sys.path.insert(0, "/opt/trn_rl_repo")

INF_MIN = -10000.0
B, S, T = 256, 1024, 128
START, END = T - 2, T - 1
SROW = 96
HALF = S // 2
NCORES = 8
BC = B // NCORES
RENORM = (256,)
NREN = len(RENORM)
NST = 1 + 2 * NREN           # stage rows: dot + fwd recips + bwd recips
FCH = 2048                   # f chunk free size (64 steps x 32 lanes)
NCHK = (HALF * BC) // FCH

_cache = {}


def _build_program(half=HALF):
    RENORM_L = tuple(r for r in RENORM if r < half)
    import concourse.bass as bass
    import concourse.mybir as mybir
    from contextlib import ExitStack

    f32 = mybir.dt.float32
    bf16 = mybir.dt.bfloat16
    AF = mybir.ActivationFunctionType
    MUL = mybir.AluOpType.mult
    NW = 3
    NV = 4

    nc = bass.Bass()
    ewf_d = nc.declare_dram_parameter("ewf", [T, T], bf16, isOutput=False)
    ewb_d = nc.declare_dram_parameter("ewb", [T, T], bf16, isOutput=False)
    ff_d = nc.declare_dram_parameter("ff", [T, HALF * BC], bf16, isOutput=False)
    fb_d = nc.declare_dram_parameter("fb", [T, HALF * BC], bf16, isOutput=False)
    res_d = nc.declare_dram_parameter("res", [1, NST * BC], f32, isOutput=True)

    # --- emission-order bookkeeping (shared ground truth between engines) ---
    # dvef incs: DVE_f(t) for t=2..HALF, +1 scale after each fwd renorm t
    # dveb incs: DVE_b(k) for k=1..HALF, +1 scale after DVE_b(kappa+1), +1 qdot
    def dvef_after(t):            # incs once DVE_f(t) emitted (no scale yet)
        return (t - 1) + sum(1 for r in RENORM_L if r < t)

    def dvef_scaled(t):           # incs incl. scale at renorm t
        return (t - 1) + sum(1 for r in RENORM_L if r <= t)

    def dveb_after(k):            # incs once DVE_b(k) emitted
        return k + sum(1 for r in RENORM_L if r + 1 < k)

    def dveb_scaled(k):           # incs incl. scale of u_k (k = kappa+1)
        return k + sum(1 for r in RENORM_L if r + 1 <= k)

    es = ExitStack()
    with es:
        ewf = es.enter_context(nc.sbuf_tensor("ewf_sb", [T, T], bf16))
        ewb = es.enter_context(nc.sbuf_tensor("ewb_sb", [T, T], bf16))
        ffc = [es.enter_context(nc.sbuf_tensor(f"ff{c}", [T, FCH], bf16))
               for c in range(NCHK)]
        fbc = [es.enter_context(nc.sbuf_tensor(f"fb{c}", [T, FCH], bf16))
               for c in range(NCHK)]
        vf = [es.enter_context(nc.sbuf_tensor(f"vf{k}", [T, BC], bf16))
              for k in range(NV)]
        ub = [es.enter_context(nc.sbuf_tensor(f"ub{k}", [T, BC], bf16))
              for k in range(NV)]
        rone = es.enter_context(nc.sbuf_tensor("rone", [T, BC], bf16))
        ones_sb = es.enter_context(nc.sbuf_tensor("ones_sb", [1, T], f32))
        rcf = [es.enter_context(nc.sbuf_tensor(f"rcf{j}", [1, BC], f32))
               for j in range(NREN)]
        rcb = [es.enter_context(nc.sbuf_tensor(f"rcb{j}", [1, BC], f32))
               for j in range(NREN)]
        qdot = es.enter_context(nc.sbuf_tensor("qdot", [T, BC], bf16))
        stage = es.enter_context(nc.sbuf_tensor("stage", [1, NST * BC], f32))
        wf = [es.enter_context(nc.psum_tensor(f"wf{k}", [T, BC], f32))
              for k in range(NW)]
        rb = [es.enter_context(nc.psum_tensor(f"rb{k}", [T, BC], f32))
              for k in range(NW)]
        bc_f = es.enter_context(nc.psum_tensor("bc_f", [T, BC], f32))
        bc_b = es.enter_context(nc.psum_tensor("bc_b", [T, BC], f32))
        s_dma = es.enter_context(nc.semaphore("s_dma"))
        s_w1 = es.enter_context(nc.semaphore("s_w1"))
        s_w2 = es.enter_context(nc.semaphore("s_w2"))
        s_c0f = es.enter_context(nc.semaphore("s_c0f"))
        s_c0b = es.enter_context(nc.semaphore("s_c0b"))
        s_ini = es.enter_context(nc.semaphore("s_ini"))
        s_pef = es.enter_context(nc.semaphore("s_pef"))
        s_dvef = es.enter_context(nc.semaphore("s_dvef"))
        s_peb = es.enter_context(nc.semaphore("s_peb"))
        s_dveb = es.enter_context(nc.semaphore("s_dveb"))
        s_rc = es.enter_context(nc.semaphore("s_rc"))
        s_bc = es.enter_context(nc.semaphore("s_bc"))
        s_fin = es.enter_context(nc.semaphore("s_fin"))
        s_out = es.enter_context(nc.semaphore("s_out"))
        block = es.enter_context(nc.Block())

        # startup DMAs (ewf, ewb, ff0, fb0) fly concurrently on dedicated
        # sems; chunks 1.. serialize on s_dma: ff1, fb1, ff2, fb2, ...
        def ff_ready(c):
            return 16 * (2 * (c - 1) + 1)

        def fb_ready(c):
            return 16 * (2 * (c - 1) + 2)

        @block.sync
        def _(sync):
            # startup: 4 concurrent DMAs, one per dedicated semaphore
            # (single inc per sem -> no completion-order race)
            sync.dma_start(ewf[:], ewf_d[:]).then_inc(s_w1, 16)
            sync.dma_start(ewb[:], ewb_d[:]).then_inc(s_w2, 16)
            sync.dma_start(ffc[0][:], ff_d[:, 0:FCH]).then_inc(s_c0f, 16)
            sync.dma_start(fbc[0][:], fb_d[:, 0:FCH]).then_inc(s_c0b, 16)
            # remaining chunks: serialized issue pins completion order so
            # the counting-semaphore thresholds are race-free
            n = 0
            for c in range(1, NCHK):
                sync.dma_start(ffc[c][:], ff_d[:, c * FCH:(c + 1) * FCH]
                               ).then_inc(s_dma, 16)
                n += 16
                sync.wait_ge(s_dma, n)
                sync.dma_start(fbc[c][:], fb_d[:, c * FCH:(c + 1) * FCH]
                               ).then_inc(s_dma, 16)
                n += 16
                sync.wait_ge(s_dma, n)
            sync.wait_ge(s_fin, 1)
            sync.dma_start(res_d[:], stage[:]).then_inc(s_out, 16)
            sync.wait_ge(s_out, 16)

        @block.vector
        def _(vector):
            vector.memset(ones_sb[:], 1.0)
            vector.memset(rone[:], 1.0).then_inc(s_ini, 1)
            rc_cnt = 0
            bc_cnt = 0
            seen_ff = -1
            seen_fb = -1
            for t in range(2, half + 1):
                k = t - 1
                # DVE_f(t): vf[t] = f_t * wf[t]
                col = (t - 1) * BC
                c0 = col // FCH
                if c0 > seen_ff:
                    # chunk-entry wait only (sticky engine wait state);
                    # avoids a redundant wait instruction per step
                    if c0 == 0:
                        vector.wait_ge(s_c0f, 16)
                    else:
                        vector.wait_ge(s_dma, ff_ready(c0))
                    seen_ff = c0
                vector.wait_ge(s_pef, t - 1)
                vector.tensor_tensor(vf[t % NV][:],
                                     ffc[c0][:, col % FCH:col % FCH + BC],
                                     wf[t % NW][:], MUL).then_inc(s_dvef, 1)
                if t in RENORM_L:
                    j = RENORM_L.index(t)
                    # recip of sum row of wf[t] (= sum of v_{t-1})
                    vector.wait_ge(s_pef, t - 1)
                    vector.reciprocal(rcf[j][:], wf[t % NW][SROW:SROW + 1, :]
                                      ).then_inc(s_rc, 1)
                    rc_cnt += 1
                    # scale vf[t] by bcast (PE emits bc_f right after MM_f(t))
                    bc_cnt += 1
                    vector.wait_ge(s_bc, bc_cnt)
                    vector.wait_ge(s_dvef, dvef_after(t))
                    vector.tensor_tensor(vf[t % NV][:], vf[t % NV][:],
                                         bc_f[:], MUL).then_inc(s_dvef, 1)
                # DVE_b(k): u_k = f_bwd(k) * r  (r = rb[k-1] PSUM, or ones)
                colb = (k - 1) * BC
                c1_ = colb // FCH
                if c1_ > seen_fb:
                    if c1_ == 0:
                        vector.wait_ge(s_c0b, 16)
                    else:
                        vector.wait_ge(s_dma, fb_ready(c1_))
                    seen_fb = c1_
                if k > 1:
                    vector.wait_ge(s_peb, k - 1)
                    rsrc = rb[(k - 1) % NW][:]
                else:
                    vector.wait_ge(s_ini, 1)
                    rsrc = rone[:]
                vector.tensor_tensor(ub[k % NV][:],
                                     fbc[c1_][:, colb % FCH:colb % FCH + BC],
                                     rsrc, MUL).then_inc(s_dveb, 1)
                if k in RENORM_L:
                    j = RENORM_L.index(k)
                    vector.wait_ge(s_peb, k)
                    vector.reciprocal(rcb[j][:], rb[k % NW][SROW:SROW + 1, :]
                                      ).then_inc(s_rc, 1)
                    rc_cnt += 1
                if (k - 1) in RENORM_L:
                    # scale u_k by bcast of 1/sum(r_{kappa}) with kappa = k-1
                    bc_cnt += 1
                    vector.wait_ge(s_bc, bc_cnt)
                    vector.wait_ge(s_dveb, dveb_after(k))
                    vector.tensor_tensor(ub[k % NV][:], ub[k % NV][:],
                                         bc_b[:], MUL).then_inc(s_dveb, 1)
            # tail: DVE_b(half), then qdot
            k = half
            colb = (k - 1) * BC
            c1_ = colb // FCH
            if colb // FCH > seen_fb:
                vector.wait_ge(s_c0b if c1_ == 0 else s_dma,
                               16 if c1_ == 0 else fb_ready(c1_))
            vector.wait_ge(s_peb, k - 1)
            vector.tensor_tensor(ub[k % NV][:],
                                 fbc[c1_][:, colb % FCH:colb % FCH + BC],
                                 rb[(k - 1) % NW][:], MUL).then_inc(s_dveb, 1)
            vector.wait_ge(s_peb, k)
            vector.wait_ge(s_dvef, dvef_scaled(half))
            vector.tensor_tensor(qdot[:], vf[half % NV][:],
                                 rb[half % NW][:], MUL).then_inc(s_dveb, 1)

        @block.tensor
        def _(pe):
            bc_cnt = 0
            rc_need = {}  # map: emission point -> rc count needed
            # rc incs in vector order: at t in RENORM: recip_f (after DVE_f);
            # at k=t-1 in RENORM: recip_b. Build the same running count:
            rc_at = {}
            rc = 0
            for t in range(2, half + 1):
                if t in RENORM_L:
                    rc += 1
                    rc_at[("f", t)] = rc
                if (t - 1) in RENORM_L:
                    rc += 1
                    rc_at[("b", t - 1)] = rc
            pe.wait_ge(s_w1, 16)
            pe.wait_ge(s_w2, 16)
            pe.wait_ge(s_ini, 1)
            for t in range(2, half + 1):
                k = t - 1
                # MM_f(t)
                if t == 2:
                    pe.wait_ge(s_c0f, 16)
                    rhs = ffc[0][:, 0:BC]
                else:
                    pe.wait_ge(s_dvef, dvef_scaled(t - 1))
                    rhs = vf[(t - 1) % NV][:]
                pe.matmul(wf[t % NW][:], lhsT=ewf[:], rhs=rhs,
                          start=True, stop=True).then_inc(s_pef, 1)
                if t in RENORM_L:
                    # bc_f right after MM_f(t): scale_vf (vec idx t) needs it
                    j = RENORM_L.index(t)
                    pe.wait_ge(s_rc, rc_at[("f", t)])
                    pe.matmul(bc_f[:], lhsT=ones_sb[:], rhs=rcf[j][:],
                              start=True, stop=True).then_inc(s_bc, 1)
                    bc_cnt += 1
                if (t - 3) in RENORM_L:
                    # bc_b for bwd renorm at kappa=t-3: after MM_b(kappa)
                    # (PE idx kappa+2), before MM_b(kappa+1) (this idx)
                    j = RENORM_L.index(t - 3)
                    pe.wait_ge(s_rc, rc_at[("b", t - 3)])
                    pe.matmul(bc_b[:], lhsT=ones_sb[:], rhs=rcb[j][:],
                              start=True, stop=True).then_inc(s_bc, 1)
                    bc_cnt += 1
                # MM_b(k2 = t-2): one-index lag so it consumes u from the
                # PREVIOUS vector index (decouples the two chains' latencies)
                k2 = t - 2
                if k2 >= 1:
                    pe.wait_ge(s_dveb, dveb_scaled(k2))
                    pe.matmul(rb[k2 % NW][:], lhsT=ewb[:], rhs=ub[k2 % NV][:],
                              start=True, stop=True).then_inc(s_peb, 1)
            # tail: MM_b(half-1), MM_b(half), final sum MM
            for k2 in (half - 1, half):
                pe.wait_ge(s_dveb, dveb_scaled(k2))
                pe.matmul(rb[k2 % NW][:], lhsT=ewb[:], rhs=ub[k2 % NV][:],
                          start=True, stop=True).then_inc(s_peb, 1)
            pe.wait_ge(s_dveb, dveb_after(half) + 1)   # qdot inc
            pe.matmul(bc_f[:], lhsT=ewf[:], rhs=qdot[:],
                      start=True, stop=True).then_inc(s_bc, 1)

        @block.scalar
        def _(scalar):
            # off-critical-path: copy recips + final dot row to stage
            rc_at = {}
            rc = 0
            order = []
            for t in range(2, half + 1):
                if t in RENORM_L:
                    rc += 1
                    rc_at[("f", t)] = rc
                    order.append(("f", t))
                if (t - 1) in RENORM_L:
                    rc += 1
                    rc_at[("b", t - 1)] = rc
                    order.append(("b", t - 1))
            for typ, idx in order:
                j = RENORM_L.index(idx)
                scalar.wait_ge(s_rc, rc_at[(typ, idx)])
                if typ == "f":
                    scalar.activation(stage[:, (1 + j) * BC:(2 + j) * BC],
                                      rcf[j][:], AF.Copy)
                else:
                    scalar.activation(stage[:, (1 + NREN + j) * BC:
                                            (2 + NREN + j) * BC],
                                      rcb[j][:], AF.Copy)
            # final: all bcasts + 1 sum MM
            scalar.wait_ge(s_bc, 2 * len(RENORM_L) + 1)
            scalar.activation(stage[:, 0:BC], bc_f[SROW:SROW + 1, :],
                              AF.Copy).then_inc(s_fin, 1)
    return nc


def _host_constants(fp, tp):
    """g (step-1 fold), mu (mean log growth), c1 (scale) — float64 on 8 lanes."""
    alpha0 = np.full(T, INF_MIN)
    alpha0[START] = 0.0
    m0 = tp + alpha0[None, :]
    gmax = m0.max(axis=1, keepdims=True)
    g = gmax[:, 0] + np.log(np.exp(m0 - gmax).sum(axis=1))

    nb = 8
    A64 = np.exp(tp)
    a = fp[:nb, 0, :] + g[None, :]
    vv = np.exp(a - a.max(axis=1, keepdims=True)).T
    ac = a.max(axis=1)
    m_first = float((np.log(vv.sum(axis=0)) + ac).mean())
    for t in range(1, S):
        vv = np.exp(fp[:nb, t, :]).T * (A64 @ vv)
        m = vv.max(axis=0)
        vv /= m[None, :]
        ac += np.log(m)
    m_last = float((np.log(vv.sum(axis=0)) + ac).mean())
    mu = (m_last - m_first) / (S - 1)
    c1 = float(g.max())
    return g, mu, c1


def run(features, batch_len, transitions, trace=False):
    from concourse.bass_utils import run_bass_kernel_spmd
    import ml_dtypes

    features = np.asarray(features, dtype=np.float32)
    batch_len = np.asarray(batch_len, dtype=np.int32)
    transitions = np.asarray(transitions, dtype=np.float32)
    bft = ml_dtypes.bfloat16

    perm = np.arange(T)
    perm[SROW], perm[END] = END, SROW
    fp = features[:, :, perm].astype(np.float64)
    tp = transitions[perm][:, perm].astype(np.float64)
    g, mu, c1 = _host_constants(fp, tp)

    A = np.exp(tp - mu)
    A[SROW, :] = 1.0
    A[:, SROW] = 0.0
    A[SROW, SROW] = 1.0
    Ab = A.copy()
    Ab[:, SROW] = 1.0
    ewf = np.ascontiguousarray(A.T).astype(bft)    # lhsT fwd: out = A @ v
    ewb = np.ascontiguousarray(Ab).astype(bft)     # lhsT bwd: out = Ab.T @ u

    blen = batch_len.astype(np.int64)
    fexp = np.exp(fp).astype(np.float32)
    fexp[:, 0, :] = np.exp(fp[:, 0, :] + g[None, :] - c1)
    dead = np.arange(S)[None, :, None] >= blen[:, None, None]
    fexp = np.where(dead, 0.0, fexp)
    fexp[:, :, SROW] = np.where(dead[:, :, 0], 1.0, 0.0)
    fexp = fexp.astype(bft)

    in_maps = []
    for cid in range(NCORES):
        fc = fexp[cid * BC:(cid + 1) * BC]              # [32, 1024, 128]
        ffwd = fc[:, :HALF, :]                          # steps 1..512
        fbwd = fc[:, :HALF - 1:-1, :]                   # steps 1024..513
        ffwd = np.ascontiguousarray(ffwd.transpose(2, 1, 0)).reshape(T, HALF * BC)
        fbwd = np.ascontiguousarray(fbwd.transpose(2, 1, 0)).reshape(T, HALF * BC)
        in_maps.append({"ewf": ewf, "ewb": ewb, "ff": ffwd, "fb": fbwd})

    if "nc" not in _cache:
        _cache["nc"] = _build_program()
    res = None
    for attempt in range(3):
        try:
            res = run_bass_kernel_spmd(_cache["nc"], in_maps,
                                       list(range(NCORES)), trace=trace)
            break
        except Exception:
            # transient backend failures (device desync) — retry
            if attempt == 2:
                raise
            import time
            time.sleep(2.0)

    out = _postprocess(res, blen, mu, c1)
    if np.isnan(out).any() or np.isinf(out).any():
        # transient device-state corruption observed to return garbage
        # without raising — rerun once on fresh buffers
        res = run_bass_kernel_spmd(_cache["nc"], in_maps,
                                   list(range(NCORES)), trace=trace)
        out = _postprocess(res, blen, mu, c1)
    return out, res


def _postprocess(res, blen, mu, c1):
    out = np.zeros(B, dtype=np.float32)
    for cid in range(NCORES):
        st = np.asarray(res.results[cid]["res"]).reshape(NST, BC
                                                         ).astype(np.float64)
        dot = st[0]
        corr = -np.log(st[1:]).sum(axis=0)   # staged values are reciprocals
        lb = blen[cid * BC:(cid + 1) * BC]
        out[cid * BC:(cid + 1) * BC] = (
            np.log(dot) + corr + c1 + (lb - 1) * mu - 10000.0
        ).astype(np.float32)
    return out


def kernel(features, batch_len, transitions):
    out, _ = run(features, batch_len, transitions, trace=False)
    return out



# revision 7
# speedup vs baseline: 145.2437x; 145.2437x over previous
"""CRF forward on 8 Trainium2 cores — segmented rank-1 scan, dead-packed.

Each lane's 1024-step linear chain v <- f_t * (A v) splits into K=64
segments of L=16 steps. Positive-matrix products contract to rank-1 at
~e^-1/step (validated 2e-8 at L=16), and dead-padded steps make segments
past batch_len EXACTLY rank-1, so only ALIVE segments are computed: the
(lane, segment) pairs are packed into ~986 columns per core (lanes are
assigned to cores by greedy load balancing on segment counts; packing is
j-major so the v1 block is the first 32 columns).  Per tick the device
advances every packed column one step: fwd probes a_j = M_j @ (v1|1) and
bwd pre-probes u_j (= M_j^T 1 short of the final A^T, applied on host).
Serial depth L=16 ticks.  Host combine per lane (truncated at its last
alive segment jm):  log s = sum_{j=1..jm} log(u_j . A a_{j-1})
- sum_{j=1..jm-1} log(sum a_j)   (jm=0: log s = log sum a_0),
out = log s + c1 + (len-1)*mu - 10000.  bf16 tiles, f32 PSUM, no renorms
(probe range ~[1e-3,1e3]).  PE: all matmuls (f leads b by one tick); DVE:
both elementwise streams (GpSimd cannot read PSUM on TRN2); SP+Act issue
the f-block DMA streams from both ends of the tick axis in parallel."""
import sys
import numpy as np

sys.path.insert(0, "/opt/trn_rl_repo")

INF_MIN = -10000.0
B, S, T = 256, 1024, 128
START, END = T - 2, T - 1
SROW = 96
NCORES = 8
LANES = 32                 # lanes per core (greedy-balanced bins of 32)
KSEG = 128                 # segments per chain
L = S // KSEG              # 8 ticks
MMW = 512                  # matmul chunk width (one PSUM bank of f32)

_cache = {}


def _build_program(WP, L):
    import concourse.bass as bass
    import concourse.mybir as mybir
    from contextlib import ExitStack

    f32 = mybir.dt.float32
    bf16 = mybir.dt.bfloat16
    MUL = mybir.AluOpType.mult
    NV = 3                                  # sbuf ping-pong depth
    NCH = (WP + MMW - 1) // MMW             # matmul chunks per row
    PW = NCH * MMW                          # psum row width (bank-aligned)
    NW = 2 if NCH <= 2 else 1               # psum ping-pong depth
    assert NCH * NW * 2 * (MMW * 4 // 2048) <= 8, "psum banks"
    chunks = [(i * MMW, min((i + 1) * MMW, WP)) for i in range(NCH)]

    nc = bass.Bass()
    ewf_d = nc.declare_dram_parameter("ewf", [T, T], bf16, isOutput=False)
    ewb_d = nc.declare_dram_parameter("ewb", [T, T], bf16, isOutput=False)
    ff_d = nc.declare_dram_parameter("ff", [T, L * WP], bf16, isOutput=False)
    res_d = nc.declare_dram_parameter("res", [T, 2 * WP], bf16, isOutput=True)

    es = ExitStack()
    with es:
        ewf = es.enter_context(nc.sbuf_tensor("ewf_sb", [T, T], bf16))
        ewb = es.enter_context(nc.sbuf_tensor("ewb_sb", [T, T], bf16))
        ffsb = es.enter_context(nc.sbuf_tensor("ffsb", [T, L * WP], bf16))
        vone = es.enter_context(nc.sbuf_tensor("vone", [T, MMW], bf16))
        vf = [es.enter_context(nc.sbuf_tensor(f"vf{k}", [T, WP], bf16))
              for k in range(NV)]
        ub = [es.enter_context(nc.sbuf_tensor(f"ub{k}", [T, WP], bf16))
              for k in range(NV)]
        wf = [es.enter_context(nc.psum_tensor(f"wf{k}", [T, PW], f32))
              for k in range(NW)]
        rb = [es.enter_context(nc.psum_tensor(f"rb{k}", [T, PW], f32))
              for k in range(NW)]
        s_w1 = es.enter_context(nc.semaphore("s_w1"))
        s_w2 = es.enter_context(nc.semaphore("s_w2"))
        s_one = es.enter_context(nc.semaphore("s_one"))
        s_bf = [es.enter_context(nc.semaphore(f"s_bf{t}")) for t in range(L)]
        s_pef = es.enter_context(nc.semaphore("s_pef"))
        s_dvef = es.enter_context(nc.semaphore("s_dvef"))
        s_peb = es.enter_context(nc.semaphore("s_peb"))
        s_dveb = es.enter_context(nc.semaphore("s_dveb"))
        s_out = es.enter_context(nc.semaphore("s_out"))
        block = es.enter_context(nc.Block())

        def fcol(tau, s, e):
            return ffsb[:, tau * WP + s: tau * WP + e]

        # ---- DMA stream F (sync/SP): ewf, blocks 0..L/2-1, a_j out
        @block.sync
        def _(sync):
            HWP = WP // 2
            sync.dma_start(ewf[:], ewf_d[:]).then_inc(s_w1, 16)
            sync.dma_start(ffsb[:, 0:WP], ff_d[:, 0:WP]
                           ).then_inc(s_bf[0], 32)
            for tau in range(1, L // 2):
                sync.dma_start(ffsb[:, tau * WP:(tau + 1) * WP],
                               ff_d[:, tau * WP:(tau + 1) * WP]
                               ).then_inc(s_bf[tau], 16)
            sync.wait_ge(s_dvef, L)
            sync.dma_start(res_d[:, 0:WP], vf[(L - 1) % NV][:]
                           ).then_inc(s_out, 16)
            sync.wait_ge(s_out, 48)

        # ---- DMA stream B (scalar/Activation): blocks L-1 .. L/2
        @block.scalar
        def _(scalar):
            HWP = WP // 2
            scalar.dma_start(ffsb[:, (L - 1) * WP:L * WP],
                             ff_d[:, (L - 1) * WP:L * WP]
                             ).then_inc(s_bf[L - 1], 16)
            scalar.dma_start(ewb[:], ewb_d[:]).then_inc(s_w2, 16)
            for tau in range(L - 2, L // 2 - 1, -1):
                scalar.dma_start(ffsb[:, tau * WP:(tau + 1) * WP],
                                 ff_d[:, tau * WP:(tau + 1) * WP]
                                 ).then_inc(s_bf[tau], 16)
            scalar.wait_ge(s_dveb, L - 1)
            scalar.dma_start(res_d[:, WP:WP + HWP],
                             ub[(L - 1) % NV][:, 0:HWP]).then_inc(s_out, 16)
            scalar.wait_ge(s_dveb, L)
            scalar.dma_start(res_d[:, WP + HWP:2 * WP],
                             ub[(L - 1) % NV][:, HWP:WP]).then_inc(s_out, 16)

        # ---- PE: all matmuls; f-chain leads b-chain by one tick
        @block.tensor
        def _(pe):
            def mm_f(tau):
                # tick 0: every column starts from ones (v1 is folded into
                # block 0's f columns on the host: f0' = v1-step / rowsums A)
                w = wf[tau % NW]
                if tau == 0:
                    pe.wait_ge(s_one, 1)
                    for s, e in chunks:
                        mm = pe.matmul(w[:, s:e], lhsT=ewf[:],
                                       rhs=vone[:, 0:e - s],
                                       start=True, stop=True)
                    mm.then_inc(s_pef, 1)
                    return
                pe.wait_ge(s_dvef, tau)
                v = vf[(tau - 1) % NV]
                for s, e in chunks:
                    mm = pe.matmul(w[:, s:e], lhsT=ewf[:], rhs=v[:, s:e],
                                   start=True, stop=True)
                mm.then_inc(s_pef, 1)

            def mm_b(tau):
                r = rb[tau % NW]
                if tau == 0:
                    # ub(0) === f block L-1: read it directly
                    pe.wait_ge(s_bf[L - 1], 16)
                    for s, e in chunks:
                        mm = pe.matmul(r[:, s:e], lhsT=ewb[:],
                                       rhs=fcol(L - 1, s, e),
                                       start=True, stop=True)
                    mm.then_inc(s_peb, 1)
                    return
                pe.wait_ge(s_dveb, tau)
                u = ub[tau % NV]
                for s, e in chunks:
                    mm = pe.matmul(r[:, s:e], lhsT=ewb[:], rhs=u[:, s:e],
                                   start=True, stop=True)
                mm.then_inc(s_peb, 1)

            pe.wait_ge(s_w1, 16)
            mm_f(0)
            pe.wait_ge(s_w2, 16)
            mm_b(0)
            mm_f(1)
            for tau in range(2, L):
                mm_f(tau)
                mm_b(tau - 1)
            # bwd MMs end at tau = L-2 (final A^T applied on host)

        # ---- DVE: both elementwise streams
        @block.vector
        def _(vector):
            vector.memset(vone[:], 1.0).then_inc(s_one, 1)
            for tau in range(L):
                vector.wait_ge(s_bf[tau], 32 if tau == 0 else 16)
                vector.wait_ge(s_pef, tau + 1)
                vector.tensor_tensor(vf[tau % NV][:], fcol(tau, 0, WP),
                                     wf[tau % NW][:, 0:WP], MUL
                                     ).then_inc(s_dvef, 1)
                if tau >= 1:
                    vector.wait_ge(s_bf[L - 1 - tau],
                                   32 if tau == L - 1 else 16)
                    vector.wait_ge(s_peb, tau)
                    if tau == L - 1:
                        # final bwd TT in halves so the ub-out DMA overlaps
                        h = WP // 2
                        vector.tensor_tensor(ub[tau % NV][:, 0:h],
                                             fcol(0, 0, h),
                                             rb[(tau - 1) % NW][:, 0:h], MUL
                                             ).then_inc(s_dveb, 1)
                        vector.tensor_tensor(ub[tau % NV][:, h:WP],
                                             fcol(0, h, WP),
                                             rb[(tau - 1) % NW][:, h:WP], MUL
                                             ).then_inc(s_dveb, 1)
                    else:
                        vector.tensor_tensor(ub[tau % NV][:],
                                             fcol(L - 1 - tau, 0, WP),
                                             rb[(tau - 1) % NW][:, 0:WP], MUL
                                             ).then_inc(s_dveb, 1)
    return nc


def _host_constants(fp, tp):
    """g (step-1 fold), mu (mean log growth), c1 (scale) — float64, 8 lanes."""
    alpha0 = np.full(T, INF_MIN)
    alpha0[START] = 0.0
    m0 = tp + alpha0[None, :]
    gmax = m0.max(axis=1, keepdims=True)
    g = gmax[:, 0] + np.log(np.exp(m0 - gmax).sum(axis=1))

    nb = 8
    A64 = np.exp(tp)
    a = fp[:nb, 0, :] + g[None, :]
    vv = np.exp(a - a.max(axis=1, keepdims=True)).T
    ac = a.max(axis=1)
    m_first = float((np.log(vv.sum(axis=0)) + ac).mean())
    for t in range(1, S):
        vv = np.exp(fp[:nb, t, :]).T * (A64 @ vv)
        m = vv.max(axis=0)
        vv /= m[None, :]
        ac += np.log(m)
    m_last = float((np.log(vv.sum(axis=0)) + ac).mean())
    mu = (m_last - m_first) / (S - 1)
    c1 = float(g.max())
    return g, mu, c1


def _layout(batch_len):
    """Greedy lane->core assignment + j-major packed column layout.

    L adapts upward (L=8 default) so that WP fits the PSUM budget
    (2 chains x WP x 4B <= 16KB/partition -> WP <= 2048)."""
    blen = batch_len.astype(np.int64)
    for Lc in (8, 16, 32, 64, 128, 256, 512, 1024):
        ks = S // Lc
        nseg = np.maximum(1, (blen - 2) // Lc + 1)
        nseg = np.where(blen == 1, 1, nseg).astype(np.int64)
        order = np.argsort(-nseg, kind="stable")
        loads = [0] * NCORES
        counts = [0] * NCORES
        core_lanes = [[] for _ in range(NCORES)]
        for lane in order:
            cands = [c for c in range(NCORES) if counts[c] < LANES]
            c = min(cands, key=lambda c: loads[c])
            loads[c] += int(nseg[lane])
            counts[c] += 1
            core_lanes[c].append(int(lane))
        for c in range(NCORES):
            core_lanes[c].sort()
        offs = []          # per core: dict[(lane, j)] -> col
        pc = []
        for c in range(NCORES):
            o = {}
            col = 0
            for j in range(ks):
                for lane in core_lanes[c]:
                    if nseg[lane] > j:
                        o[(lane, j)] = col
                        col += 1
            offs.append(o)
            pc.append(col)
        WP = ((max(pc) + 31) // 32) * 32
        if WP <= 2048:
            return core_lanes, offs, nseg, WP, Lc
    raise AssertionError("no feasible L")


def _prep_inputs(features, batch_len, transitions):
    import ml_dtypes
    bft = ml_dtypes.bfloat16

    perm = np.arange(T)
    perm[SROW], perm[END] = END, SROW
    fp = features[:, :, perm].astype(np.float64)
    tp = transitions[perm][:, perm].astype(np.float64)
    g, mu, c1 = _host_constants(fp, tp)

    A = np.exp(tp - mu)
    A[SROW, :] = 1.0
    A[:, SROW] = 0.0
    A[SROW, SROW] = 1.0
    ewf = np.ascontiguousarray(A.T).astype(bft)   # lhsT fwd: out = A @ v
    ewb = np.ascontiguousarray(A).astype(bft)     # lhsT bwd: out = A.T @ u

    blen = batch_len.astype(np.int64)
    fexp = np.exp(fp).astype(np.float32)
    fexp[:, 0, :] = np.exp(fp[:, 0, :] + g[None, :] - c1)
    dead = np.arange(S)[None, :, None] >= blen[:, None, None]
    fexp = np.where(dead, 0.0, fexp)
    fexp[:, :, SROW] = np.where(dead[:, :, 0], 1.0, 0.0)
    fexp = fexp.astype(bft)
    deadcol = np.zeros((B, 1, T), dtype=bft)
    deadcol[:, 0, SROW] = 1.0
    # matmul step m uses emission col m+1; pad a virtual dead step at m=S-1
    fm = np.concatenate([fexp[:, 1:, :], deadcol], axis=1)  # [B, S, T]

    core_lanes, offs, nseg, WP, Lc = _layout(batch_len)
    ks = S // Lc
    pad_col = np.zeros(T, dtype=bft)
    pad_col[SROW] = 1.0

    # fold v1 into segment 0's first f column so every packed column can
    # start from ones on-device:  f0' = f_{m0} * (A v1) / (A 1)
    Abf = A.astype(bft).astype(np.float64)
    r0 = Abf.sum(axis=1)                                    # A @ 1
    v1all = np.exp(fp[:, 0, :] + g[None, :] - c1)           # [B, T] float64
    Av1 = v1all @ Abf.T                                     # (A @ v1) rows
    f0p = (fm[:, 0, :].astype(np.float64) * Av1 / r0[None, :]).astype(bft)

    in_maps = []
    for cid in range(NCORES):
        ff = np.empty((T, Lc, WP), dtype=bft)
        ff[:] = pad_col[:, None, None]
        # packed columns: value at block b = fm[lane, j*Lc+b, :]
        lanes_j = [[] for _ in range(ks)]
        for (lane, j), col in offs[cid].items():
            lanes_j[j].append((col, lane))
        for j in range(ks):
            if not lanes_j[j]:
                continue
            cols = np.array([c for c, _ in lanes_j[j]])
            ls = np.array([ln for _, ln in lanes_j[j]])
            ff[:, :, cols] = fm[ls, j * Lc:(j + 1) * Lc, :].transpose(2, 1, 0)
            if j == 0:
                ff[:, 0, cols] = f0p[ls].T
        in_maps.append({"ewf": ewf, "ewb": ewb,
                        "ff": np.ascontiguousarray(ff).reshape(T, Lc * WP)})
    meta = (core_lanes, offs, nseg, WP, Lc)
    return in_maps, A, blen, mu, c1, meta


def _postprocess(res, A, blen, mu, c1, meta):
    core_lanes, offs, nseg, WP, Lc = meta
    out = np.zeros(B, dtype=np.float32)
    for cid in range(NCORES):
        st = np.asarray(res.results[cid]["res"]).astype(np.float64)
        a = st[:, 0:WP]
        u = st[:, WP:2 * WP]
        Aa = A @ a
        o = offs[cid]
        for lane in core_lanes[cid]:
            jm = int(nseg[lane]) - 1
            if jm == 0:
                logs = np.log(a[:, o[(lane, 0)]].sum())
            else:
                dsum = 0.0
                nsum = 0.0
                for j in range(1, jm + 1):
                    dsum += np.log(np.dot(u[:, o[(lane, j)]],
                                          Aa[:, o[(lane, j - 1)]]))
                    if j <= jm - 1:
                        nsum += np.log(a[:, o[(lane, j)]].sum())
                logs = dsum - nsum
            out[lane] = np.float32(
                logs + c1 + (blen[lane] - 1) * mu - 10000.0)
    return out


def run(features, batch_len, transitions, trace=False):
    from concourse.bass_utils import run_bass_kernel_spmd

    features = np.asarray(features, dtype=np.float32)
    batch_len = np.asarray(batch_len, dtype=np.int32)
    transitions = np.asarray(transitions, dtype=np.float32)

    in_maps, A, blen, mu, c1, meta = _prep_inputs(
        features, batch_len, transitions)
    WP, Lc = meta[3], meta[4]
    key = ("nc", WP, Lc)
    if key not in _cache:
        _cache[key] = _build_program(WP, Lc)
    res = None
    for attempt in range(3):
        try:
            res = run_bass_kernel_spmd(_cache[key], in_maps,
                                       list(range(NCORES)), trace=trace)
            break
        except Exception:
            if attempt == 2:
                raise
            import time
            time.sleep(2.0)

    out = _postprocess(res, A, blen, mu, c1, meta)
    if np.isnan(out).any() or np.isinf(out).any():
        res = run_bass_kernel_spmd(_cache[key], in_maps,
                                   list(range(NCORES)), trace=trace)
        out = _postprocess(res, A, blen, mu, c1, meta)
    return out, res


def kernel(features, batch_len, transitions):
    out, _ = run(features, batch_len, transitions, trace=False)
    return out


# revision 16
# speedup vs baseline: 148.1042x; 1.0197x over previous
"""CRF forward on 8 Trainium2 cores — segmented rank-1 scan, dead-packed.

Each lane's 1024-step linear chain v <- f_t * (A v) splits into K=S/L
segments of L steps (L=8 by default, auto-doubled until the packed width
fits PSUM).  Positive-matrix products contract to rank-1 at ~e^-1/step
(validated: 2e-8 rel err at L=8 in float64), and dead-padded steps make
segments past batch_len EXACTLY rank-1, so only ALIVE segments are
computed: the (lane, segment) pairs are packed into ~1955 columns per
core (lanes are assigned to cores by greedy load balancing on segment
counts — near-perfect balance; packing is j-major).  Per tick the device
advances every packed column one step: fwd probes a_j = M_j @ 1 (v1 is
folded into segment 0's first f column: f0' = f_m0 * (A v1) / (A 1)) and
bwd pre-probes u_j (= M_j^T 1 short of the final A^T, applied on host in
float64).  Serial depth: L=8 ticks instead of 1024 steps.  Host combine
per lane, truncated at its last alive segment jm:
  log s = sum_{j=1..jm} log(u_j . A a_{j-1}) - sum_{j=1..jm-1} log(sum a_j)
  (jm=0: log s = log sum a_0),   out = log s + c1 + (len-1)*mu - 10000.
bf16 tiles (f data in fp8e4m3 — halves DMA; rel err 8.7e-5, gate 2e-2),
f32 PSUM, no renorms (probe range ~[1e-3,1e3], A scaled by e^-mu).  Engine layout: PE runs all matmuls in 512-wide PSUM-bank chunks
(f-chain leads b-chain by one tick in program order); DVE runs both
elementwise streams (it is the only engine that can read PSUM — GpSimd
cannot, Activation has no tensor_tensor — and is the 33us steady-state
bottleneck at ~1.1ns/col for bf16*f32psum); SP and Activation issue the
f-block DMA streams from both ends of the tick axis in parallel; the
final bwd TT is split in halves so the result DMAs overlap the tail.
Measured: ~50.0us device exec (NTFF) vs 358us for the previous
meet-in-the-middle 512-step chain kernel."""
import sys
import numpy as np

sys.path.insert(0, "/opt/trn_rl_repo")

INF_MIN = -10000.0
B, S, T = 256, 1024, 128
START, END = T - 2, T - 1
SROW = 96
NCORES = 8
LANES = 32                 # lanes per core (greedy-balanced bins of 32)
KSEG = 128                 # segments per chain
L = S // KSEG              # 8 ticks
MMW = 512                  # matmul chunk width (one PSUM bank of f32)

_cache = {}


def _build_program(WP, L):
    import concourse.bass as bass
    import concourse.mybir as mybir
    from contextlib import ExitStack

    f32 = mybir.dt.float32
    bf16 = mybir.dt.bfloat16
    fp8 = mybir.dt.float8e4
    MUL = mybir.AluOpType.mult
    NV = 3                                  # sbuf ping-pong depth
    NCH = (WP + MMW - 1) // MMW             # matmul chunks per row
    PW = NCH * MMW                          # psum row width (bank-aligned)
    NW = 2 if NCH <= 2 else 1               # psum ping-pong depth
    assert NCH * NW * 2 * (MMW * 4 // 2048) <= 8, "psum banks"
    chunks = [(i * MMW, min((i + 1) * MMW, WP)) for i in range(NCH)]

    nc = bass.Bass()
    ewf_d = nc.declare_dram_parameter("ewf", [T, T], bf16, isOutput=False)
    ewb_d = nc.declare_dram_parameter("ewb", [T, T], bf16, isOutput=False)
    ff_d = nc.declare_dram_parameter("ff", [T, L * WP], fp8, isOutput=False)
    res_d = nc.declare_dram_parameter("res", [T, 2 * WP], bf16, isOutput=True)

    es = ExitStack()
    with es:
        ewf = es.enter_context(nc.sbuf_tensor("ewf_sb", [T, T], bf16))
        ewb = es.enter_context(nc.sbuf_tensor("ewb_sb", [T, T], bf16))
        ffsb = es.enter_context(nc.sbuf_tensor("ffsb", [T, L * WP], fp8))
        vone = es.enter_context(nc.sbuf_tensor("vone", [T, MMW], bf16))
        vf = [es.enter_context(nc.sbuf_tensor(f"vf{k}", [T, WP], bf16))
              for k in range(NV)]
        ub = [es.enter_context(nc.sbuf_tensor(f"ub{k}", [T, WP], bf16))
              for k in range(NV)]
        wf = [es.enter_context(nc.psum_tensor(f"wf{k}", [T, PW], f32))
              for k in range(NW)]
        rb = [es.enter_context(nc.psum_tensor(f"rb{k}", [T, PW], f32))
              for k in range(NW)]
        s_w1 = es.enter_context(nc.semaphore("s_w1"))
        s_w2 = es.enter_context(nc.semaphore("s_w2"))
        s_one = es.enter_context(nc.semaphore("s_one"))
        s_bf = [es.enter_context(nc.semaphore(f"s_bf{t}")) for t in range(L)]
        s_pef = es.enter_context(nc.semaphore("s_pef"))
        s_dvef = es.enter_context(nc.semaphore("s_dvef"))
        s_peb = es.enter_context(nc.semaphore("s_peb"))
        s_dveb = es.enter_context(nc.semaphore("s_dveb"))
        s_out = es.enter_context(nc.semaphore("s_out"))
        block = es.enter_context(nc.Block())

        def fcol(tau, s, e):
            return ffsb[:, tau * WP + s: tau * WP + e]

        # ---- DMA stream F (sync/SP): ewf, blocks 0..L/2-1, a_j out
        @block.sync
        def _(sync):
            HWP = WP // 2
            sync.dma_start(ewf[:], ewf_d[:]).then_inc(s_w1, 16)
            sync.dma_start(ffsb[:, 0:WP], ff_d[:, 0:WP]
                           ).then_inc(s_bf[0], 32)
            for tau in range(1, L // 2):
                sync.dma_start(ffsb[:, tau * WP:(tau + 1) * WP],
                               ff_d[:, tau * WP:(tau + 1) * WP]
                               ).then_inc(s_bf[tau], 16)
            sync.wait_ge(s_dvef, L)
            sync.dma_start(res_d[:, 0:WP], vf[(L - 1) % NV][:]
                           ).then_inc(s_out, 16)
            sync.wait_ge(s_out, 48)

        # ---- DMA stream B (scalar/Activation): blocks L-1 .. L/2
        @block.scalar
        def _(scalar):
            HWP = WP // 2
            scalar.dma_start(ffsb[:, (L - 1) * WP:L * WP],
                             ff_d[:, (L - 1) * WP:L * WP]
                             ).then_inc(s_bf[L - 1], 16)
            scalar.dma_start(ewb[:], ewb_d[:]).then_inc(s_w2, 16)
            for tau in range(L - 2, L // 2 - 1, -1):
                scalar.dma_start(ffsb[:, tau * WP:(tau + 1) * WP],
                                 ff_d[:, tau * WP:(tau + 1) * WP]
                                 ).then_inc(s_bf[tau], 16)
            scalar.wait_ge(s_dveb, L - 1)
            scalar.dma_start(res_d[:, WP:WP + HWP],
                             ub[(L - 1) % NV][:, 0:HWP]).then_inc(s_out, 16)
            scalar.wait_ge(s_dveb, L)
            scalar.dma_start(res_d[:, WP + HWP:2 * WP],
                             ub[(L - 1) % NV][:, HWP:WP]).then_inc(s_out, 16)

        # ---- PE: all matmuls; f-chain leads b-chain by one tick
        @block.tensor
        def _(pe):
            def mm_f(tau):
                # tick 0: every column starts from ones (v1 is folded into
                # block 0's f columns on the host: f0' = v1-step / rowsums A)
                w = wf[tau % NW]
                if tau == 0:
                    pe.wait_ge(s_one, 1)
                    for s, e in chunks:
                        mm = pe.matmul(w[:, s:e], lhsT=ewf[:],
                                       rhs=vone[:, 0:e - s],
                                       start=True, stop=True)
                    mm.then_inc(s_pef, 1)
                    return
                pe.wait_ge(s_dvef, tau)
                v = vf[(tau - 1) % NV]
                for s, e in chunks:
                    mm = pe.matmul(w[:, s:e], lhsT=ewf[:], rhs=v[:, s:e],
                                   start=True, stop=True)
                mm.then_inc(s_pef, 1)

            def mm_b(tau):
                r = rb[tau % NW]
                if tau == 0:
                    # ub(0) === f block L-1: read it directly
                    pe.wait_ge(s_bf[L - 1], 16)
                    for s, e in chunks:
                        mm = pe.matmul(r[:, s:e], lhsT=ewb[:],
                                       rhs=fcol(L - 1, s, e),
                                       start=True, stop=True)
                    mm.then_inc(s_peb, 1)
                    return
                pe.wait_ge(s_dveb, tau)
                u = ub[tau % NV]
                for s, e in chunks:
                    mm = pe.matmul(r[:, s:e], lhsT=ewb[:], rhs=u[:, s:e],
                                   start=True, stop=True)
                mm.then_inc(s_peb, 1)

            pe.wait_ge(s_w1, 16)
            mm_f(0)
            pe.wait_ge(s_w2, 16)
            mm_b(0)
            mm_f(1)
            for tau in range(2, L):
                mm_f(tau)
                mm_b(tau - 1)
            # bwd MMs end at tau = L-2 (final A^T applied on host)

        # ---- DVE: both elementwise streams
        @block.vector
        def _(vector):
            vector.memset(vone[:], 1.0).then_inc(s_one, 1)
            for tau in range(L):
                vector.wait_ge(s_bf[tau], 32 if tau == 0 else 16)
                vector.wait_ge(s_pef, tau + 1)
                vector.tensor_tensor(vf[tau % NV][:], fcol(tau, 0, WP),
                                     wf[tau % NW][:, 0:WP], MUL
                                     ).then_inc(s_dvef, 1)
                if tau >= 1:
                    vector.wait_ge(s_bf[L - 1 - tau],
                                   32 if tau == L - 1 else 16)
                    vector.wait_ge(s_peb, tau)
                    if tau == L - 1:
                        # final bwd TT in halves so the ub-out DMA overlaps
                        h = WP // 2
                        vector.tensor_tensor(ub[tau % NV][:, 0:h],
                                             fcol(0, 0, h),
                                             rb[(tau - 1) % NW][:, 0:h], MUL
                                             ).then_inc(s_dveb, 1)
                        vector.tensor_tensor(ub[tau % NV][:, h:WP],
                                             fcol(0, h, WP),
                                             rb[(tau - 1) % NW][:, h:WP], MUL
                                             ).then_inc(s_dveb, 1)
                    else:
                        vector.tensor_tensor(ub[tau % NV][:],
                                             fcol(L - 1 - tau, 0, WP),
                                             rb[(tau - 1) % NW][:, 0:WP], MUL
                                             ).then_inc(s_dveb, 1)
    return nc


def _host_constants(fp, tp):
    """g (step-1 fold), mu (mean log growth), c1 (scale) — float64, 8 lanes."""
    alpha0 = np.full(T, INF_MIN)
    alpha0[START] = 0.0
    m0 = tp + alpha0[None, :]
    gmax = m0.max(axis=1, keepdims=True)
    g = gmax[:, 0] + np.log(np.exp(m0 - gmax).sum(axis=1))

    nb = 8
    A64 = np.exp(tp)
    a = fp[:nb, 0, :] + g[None, :]
    vv = np.exp(a - a.max(axis=1, keepdims=True)).T
    ac = a.max(axis=1)
    m_first = float((np.log(vv.sum(axis=0)) + ac).mean())
    for t in range(1, S):
        vv = np.exp(fp[:nb, t, :]).T * (A64 @ vv)
        m = vv.max(axis=0)
        vv /= m[None, :]
        ac += np.log(m)
    m_last = float((np.log(vv.sum(axis=0)) + ac).mean())
    mu = (m_last - m_first) / (S - 1)
    c1 = float(g.max())
    return g, mu, c1


def _layout(batch_len):
    """Greedy lane->core assignment + j-major packed column layout.

    L adapts upward (L=8 default) so that WP fits the PSUM budget
    (2 chains x WP x 4B <= 16KB/partition -> WP <= 2048)."""
    blen = batch_len.astype(np.int64)
    for Lc in (8, 16, 32, 64, 128, 256, 512, 1024):
        ks = S // Lc
        nseg = np.maximum(1, (blen - 2) // Lc + 1)
        nseg = np.where(blen == 1, 1, nseg).astype(np.int64)
        order = np.argsort(-nseg, kind="stable")
        loads = [0] * NCORES
        counts = [0] * NCORES
        core_lanes = [[] for _ in range(NCORES)]
        for lane in order:
            cands = [c for c in range(NCORES) if counts[c] < LANES]
            c = min(cands, key=lambda c: loads[c])
            loads[c] += int(nseg[lane])
            counts[c] += 1
            core_lanes[c].append(int(lane))
        for c in range(NCORES):
            core_lanes[c].sort()
        offs = []          # per core: dict[(lane, j)] -> col
        pc = []
        for c in range(NCORES):
            o = {}
            col = 0
            for j in range(ks):
                for lane in core_lanes[c]:
                    if nseg[lane] > j:
                        o[(lane, j)] = col
                        col += 1
            offs.append(o)
            pc.append(col)
        WP = ((max(pc) + 31) // 32) * 32
        if WP <= 2048:
            return core_lanes, offs, nseg, WP, Lc
    raise AssertionError("no feasible L")


def _prep_inputs(features, batch_len, transitions):
    import ml_dtypes
    bft = ml_dtypes.bfloat16
    f8 = ml_dtypes.float8_e4m3

    perm = np.arange(T)
    perm[SROW], perm[END] = END, SROW
    fp = features[:, :, perm].astype(np.float64)
    tp = transitions[perm][:, perm].astype(np.float64)
    g, mu, c1 = _host_constants(fp, tp)

    A = np.exp(tp - mu)
    A[SROW, :] = 1.0
    A[:, SROW] = 0.0
    A[SROW, SROW] = 1.0
    ewf = np.ascontiguousarray(A.T).astype(bft)   # lhsT fwd: out = A @ v
    ewb = np.ascontiguousarray(A).astype(bft)     # lhsT bwd: out = A.T @ u

    blen = batch_len.astype(np.int64)
    fexp = np.exp(fp).astype(np.float32)
    fexp[:, 0, :] = np.exp(fp[:, 0, :] + g[None, :] - c1)
    dead = np.arange(S)[None, :, None] >= blen[:, None, None]
    fexp = np.where(dead, 0.0, fexp)
    fexp[:, :, SROW] = np.where(dead[:, :, 0], 1.0, 0.0)
    fexp = fexp.astype(bft)
    deadcol = np.zeros((B, 1, T), dtype=bft)
    deadcol[:, 0, SROW] = 1.0
    # matmul step m uses emission col m+1; pad a virtual dead step at m=S-1
    fm = np.concatenate([fexp[:, 1:, :], deadcol], axis=1)  # [B, S, T]

    core_lanes, offs, nseg, WP, Lc = _layout(batch_len)
    ks = S // Lc
    pad_col = np.zeros(T, dtype=bft)
    pad_col[SROW] = 1.0

    # fold v1 into segment 0's first f column so every packed column can
    # start from ones on-device:  f0' = f_{m0} * (A v1) / (A 1)
    Abf = A.astype(bft).astype(np.float64)
    r0 = Abf.sum(axis=1)                                    # A @ 1
    v1all = np.exp(fp[:, 0, :] + g[None, :] - c1)           # [B, T] float64
    Av1 = v1all @ Abf.T                                     # (A @ v1) rows
    f0p = (fm[:, 0, :].astype(np.float64) * Av1 / r0[None, :]).astype(bft)

    in_maps = []
    for cid in range(NCORES):
        ff = np.empty((T, Lc, WP), dtype=f8)
        ff[:] = pad_col.astype(f8)[:, None, None]
        # packed columns: value at block b = fm[lane, j*Lc+b, :]
        lanes_j = [[] for _ in range(ks)]
        for (lane, j), col in offs[cid].items():
            lanes_j[j].append((col, lane))
        for j in range(ks):
            if not lanes_j[j]:
                continue
            cols = np.array([c for c, _ in lanes_j[j]])
            ls = np.array([ln for _, ln in lanes_j[j]])
            ff[:, :, cols] = fm[ls, j * Lc:(j + 1) * Lc, :].transpose(2, 1, 0)
            if j == 0:
                ff[:, 0, cols] = f0p[ls].T
        in_maps.append({"ewf": ewf, "ewb": ewb,
                        "ff": np.ascontiguousarray(ff).reshape(T, Lc * WP)})
    meta = (core_lanes, offs, nseg, WP, Lc)
    return in_maps, A, blen, mu, c1, meta


def _postprocess(res, A, blen, mu, c1, meta):
    core_lanes, offs, nseg, WP, Lc = meta
    out = np.zeros(B, dtype=np.float32)
    for cid in range(NCORES):
        st = np.asarray(res.results[cid]["res"]).astype(np.float64)
        a = st[:, 0:WP]
        u = st[:, WP:2 * WP]
        Aa = A @ a
        o = offs[cid]
        for lane in core_lanes[cid]:
            jm = int(nseg[lane]) - 1
            if jm == 0:
                logs = np.log(a[:, o[(lane, 0)]].sum())
            else:
                dsum = 0.0
                nsum = 0.0
                for j in range(1, jm + 1):
                    dsum += np.log(np.dot(u[:, o[(lane, j)]],
                                          Aa[:, o[(lane, j - 1)]]))
                    if j <= jm - 1:
                        nsum += np.log(a[:, o[(lane, j)]].sum())
                logs = dsum - nsum
            out[lane] = np.float32(
                logs + c1 + (blen[lane] - 1) * mu - 10000.0)
    return out


def run(features, batch_len, transitions, trace=False):
    from concourse.bass_utils import run_bass_kernel_spmd

    features = np.asarray(features, dtype=np.float32)
    batch_len = np.asarray(batch_len, dtype=np.int32)
    transitions = np.asarray(transitions, dtype=np.float32)

    in_maps, A, blen, mu, c1, meta = _prep_inputs(
        features, batch_len, transitions)
    WP, Lc = meta[3], meta[4]
    key = ("nc", WP, Lc)
    if key not in _cache:
        _cache[key] = _build_program(WP, Lc)
    res = None
    for attempt in range(3):
        try:
            res = run_bass_kernel_spmd(_cache[key], in_maps,
                                       list(range(NCORES)), trace=trace)
            break
        except Exception:
            if attempt == 2:
                raise
            import time
            time.sleep(2.0)

    out = _postprocess(res, A, blen, mu, c1, meta)
    if np.isnan(out).any() or np.isinf(out).any():
        res = run_bass_kernel_spmd(_cache[key], in_maps,
                                   list(range(NCORES)), trace=trace)
        out = _postprocess(res, A, blen, mu, c1, meta)
    return out, res


def kernel(features, batch_len, transitions):
    out, _ = run(features, batch_len, transitions, trace=False)
    return out


# revision 18
# speedup vs baseline: 155.6542x; 1.0510x over previous
"""CRF forward on 8 Trainium2 cores — segmented rank-1 scan, dead-packed.

Each lane's 1024-step linear chain v <- f_t * (A v) splits into K=S/L
segments of L steps (L=8 by default, auto-doubled until the packed width
fits PSUM).  Positive-matrix products contract to rank-1 at ~e^-1/step
(validated: 2e-8 rel err at L=8 in float64), and dead-padded steps make
segments past batch_len EXACTLY rank-1, so only ALIVE segments are
computed: the (lane, segment) pairs are packed into ~1955 columns per
core (lanes are assigned to cores by greedy load balancing on segment
counts — near-perfect balance; packing is j-major).  Per tick the device
advances every packed column one step: fwd probes a_j = M_j @ 1 (v1 is
folded into segment 0's first f column: f0' = f_m0 * (A v1) / (A 1)) and
bwd pre-probes u_j (= M_j^T 1 short of the final A^T, applied on host in
float64).  Serial depth: L=8 ticks instead of 1024 steps.  Host combine
per lane, truncated at its last alive segment jm:
  log s = sum_{j=1..jm} log(u_j . A a_{j-1}) - sum_{j=1..jm-1} log(sum a_j)
  (jm=0: log s = log sum a_0),   out = log s + c1 + (len-1)*mu - 10000.
bf16 tiles (f data in fp8e4m3 — halves DMA; rel err 8.7e-5, gate 2e-2),
f32 PSUM, no renorms (probe range ~[1e-3,1e3], A scaled by e^-mu).  Engine layout: PE runs all matmuls in 512-wide PSUM-bank chunks
(f-chain leads b-chain by one tick in program order); DVE runs both
elementwise streams (it is the only engine that can read PSUM — GpSimd
cannot, Activation has no tensor_tensor — and is the 33us steady-state
bottleneck at ~1.1ns/col for bf16*f32psum); SP and Activation issue the
f-block DMA streams from both ends of the tick axis in parallel; PE and
DVE emission strictly alternates the two chains (f0,b0,f1,b1,... /
f0,(b1,f1),(b2,f2),...) so neither engine queues a stalled op ahead of a
runnable one; both final TTs are split in halves so all four result DMAs
overlap the tail compute.  Measured: ~48.5-49.6us device exec (NTFF) vs
358us for the previous meet-in-the-middle 512-step chain kernel."""
import sys
import numpy as np

sys.path.insert(0, "/opt/trn_rl_repo")

INF_MIN = -10000.0
B, S, T = 256, 1024, 128
START, END = T - 2, T - 1
SROW = 96
NCORES = 8
LANES = 32                 # lanes per core (greedy-balanced bins of 32)
KSEG = 128                 # segments per chain
L = S // KSEG              # 8 ticks
MMW = 512                  # matmul chunk width (one PSUM bank of f32)

_cache = {}


def _build_program(WP, L):
    import concourse.bass as bass
    import concourse.mybir as mybir
    from contextlib import ExitStack

    f32 = mybir.dt.float32
    bf16 = mybir.dt.bfloat16
    fp8 = mybir.dt.float8e4
    MUL = mybir.AluOpType.mult
    NV = 3                                  # sbuf ping-pong depth
    NCH = (WP + MMW - 1) // MMW             # matmul chunks per row
    PW = NCH * MMW                          # psum row width (bank-aligned)
    NW = 2 if NCH <= 2 else 1               # psum ping-pong depth
    assert NCH * NW * 2 * (MMW * 4 // 2048) <= 8, "psum banks"
    chunks = [(i * MMW, min((i + 1) * MMW, WP)) for i in range(NCH)]

    nc = bass.Bass()
    ewf_d = nc.declare_dram_parameter("ewf", [T, T], bf16, isOutput=False)
    ewb_d = nc.declare_dram_parameter("ewb", [T, T], bf16, isOutput=False)
    ff_d = nc.declare_dram_parameter("ff", [T, L * WP], fp8, isOutput=False)
    res_d = nc.declare_dram_parameter("res", [T, 2 * WP], bf16, isOutput=True)

    es = ExitStack()
    with es:
        ewf = es.enter_context(nc.sbuf_tensor("ewf_sb", [T, T], bf16))
        ewb = es.enter_context(nc.sbuf_tensor("ewb_sb", [T, T], bf16))
        ffsb = es.enter_context(nc.sbuf_tensor("ffsb", [T, L * WP], fp8))
        vone = es.enter_context(nc.sbuf_tensor("vone", [T, MMW], bf16))
        vf = [es.enter_context(nc.sbuf_tensor(f"vf{k}", [T, WP], bf16))
              for k in range(NV)]
        ub = [es.enter_context(nc.sbuf_tensor(f"ub{k}", [T, WP], bf16))
              for k in range(NV)]
        wf = [es.enter_context(nc.psum_tensor(f"wf{k}", [T, PW], f32))
              for k in range(NW)]
        rb = [es.enter_context(nc.psum_tensor(f"rb{k}", [T, PW], f32))
              for k in range(NW)]
        s_w1 = es.enter_context(nc.semaphore("s_w1"))
        s_w2 = es.enter_context(nc.semaphore("s_w2"))
        s_one = es.enter_context(nc.semaphore("s_one"))
        s_bf = [es.enter_context(nc.semaphore(f"s_bf{t}")) for t in range(L)]
        s_pef = es.enter_context(nc.semaphore("s_pef"))
        s_dvef = es.enter_context(nc.semaphore("s_dvef"))
        s_peb = es.enter_context(nc.semaphore("s_peb"))
        s_dveb = es.enter_context(nc.semaphore("s_dveb"))
        s_out = es.enter_context(nc.semaphore("s_out"))
        block = es.enter_context(nc.Block())

        def fcol(tau, s, e):
            return ffsb[:, tau * WP + s: tau * WP + e]

        # ---- DMA stream F (sync/SP): ewf, blocks 0..L/2-1, a_j out
        @block.sync
        def _(sync):
            HWP = WP // 2
            sync.dma_start(ewf[:], ewf_d[:]).then_inc(s_w1, 16)
            sync.dma_start(ffsb[:, 0:WP], ff_d[:, 0:WP]
                           ).then_inc(s_bf[0], 32)
            for tau in range(1, L // 2):
                sync.dma_start(ffsb[:, tau * WP:(tau + 1) * WP],
                               ff_d[:, tau * WP:(tau + 1) * WP]
                               ).then_inc(s_bf[tau], 16)
            sync.wait_ge(s_dvef, L)
            sync.dma_start(res_d[:, 0:HWP], vf[(L - 1) % NV][:, 0:HWP]
                           ).then_inc(s_out, 16)
            sync.wait_ge(s_dvef, L + 1)
            sync.dma_start(res_d[:, HWP:WP], vf[(L - 1) % NV][:, HWP:WP]
                           ).then_inc(s_out, 16)
            sync.wait_ge(s_out, 64)

        # ---- DMA stream B (scalar/Activation): blocks L-1 .. L/2
        @block.scalar
        def _(scalar):
            HWP = WP // 2
            scalar.dma_start(ffsb[:, (L - 1) * WP:L * WP],
                             ff_d[:, (L - 1) * WP:L * WP]
                             ).then_inc(s_bf[L - 1], 16)
            scalar.dma_start(ewb[:], ewb_d[:]).then_inc(s_w2, 16)
            for tau in range(L - 2, L // 2 - 1, -1):
                scalar.dma_start(ffsb[:, tau * WP:(tau + 1) * WP],
                                 ff_d[:, tau * WP:(tau + 1) * WP]
                                 ).then_inc(s_bf[tau], 16)
            scalar.wait_ge(s_dveb, L - 1)
            scalar.dma_start(res_d[:, WP:WP + HWP],
                             ub[(L - 1) % NV][:, 0:HWP]).then_inc(s_out, 16)
            scalar.wait_ge(s_dveb, L)
            scalar.dma_start(res_d[:, WP + HWP:2 * WP],
                             ub[(L - 1) % NV][:, HWP:WP]).then_inc(s_out, 16)

        # ---- PE: all matmuls; f-chain leads b-chain by one tick
        @block.tensor
        def _(pe):
            def mm_f(tau):
                # tick 0: every column starts from ones (v1 is folded into
                # block 0's f columns on the host: f0' = v1-step / rowsums A)
                w = wf[tau % NW]
                if tau == 0:
                    pe.wait_ge(s_one, 1)
                    for s, e in chunks:
                        mm = pe.matmul(w[:, s:e], lhsT=ewf[:],
                                       rhs=vone[:, 0:e - s],
                                       start=True, stop=True)
                    mm.then_inc(s_pef, 1)
                    return
                pe.wait_ge(s_dvef, tau)
                v = vf[(tau - 1) % NV]
                for s, e in chunks:
                    mm = pe.matmul(w[:, s:e], lhsT=ewf[:], rhs=v[:, s:e],
                                   start=True, stop=True)
                mm.then_inc(s_pef, 1)

            def mm_b(tau):
                r = rb[tau % NW]
                if tau == 0:
                    # ub(0) === f block L-1: read it directly
                    pe.wait_ge(s_bf[L - 1], 16)
                    for s, e in chunks:
                        mm = pe.matmul(r[:, s:e], lhsT=ewb[:],
                                       rhs=fcol(L - 1, s, e),
                                       start=True, stop=True)
                    mm.then_inc(s_peb, 1)
                    return
                pe.wait_ge(s_dveb, tau)
                u = ub[tau % NV]
                for s, e in chunks:
                    mm = pe.matmul(r[:, s:e], lhsT=ewb[:], rhs=u[:, s:e],
                                   start=True, stop=True)
                mm.then_inc(s_peb, 1)

            pe.wait_ge(s_w1, 16)
            mm_f(0)
            pe.wait_ge(s_w2, 16)
            for tau in range(1, L):
                mm_b(tau - 1)
                mm_f(tau)
            # bwd MMs end at tau = L-2 (final A^T applied on host)

        # ---- DVE: both elementwise streams
        @block.vector
        def _(vector):
            vector.memset(vone[:], 1.0).then_inc(s_one, 1)
            h = WP // 2
            vector.wait_ge(s_bf[0], 32)
            vector.wait_ge(s_pef, 1)
            vector.tensor_tensor(vf[0][:], fcol(0, 0, WP), wf[0][:, 0:WP],
                                 MUL).then_inc(s_dvef, 1)
            for tau in range(1, L - 1):
                vector.wait_ge(s_bf[L - 1 - tau], 16)
                vector.wait_ge(s_peb, tau)
                vector.tensor_tensor(ub[tau % NV][:],
                                     fcol(L - 1 - tau, 0, WP),
                                     rb[(tau - 1) % NW][:, 0:WP], MUL
                                     ).then_inc(s_dveb, 1)
                vector.wait_ge(s_bf[tau], 16)
                vector.wait_ge(s_pef, tau + 1)
                vector.tensor_tensor(vf[tau % NV][:], fcol(tau, 0, WP),
                                     wf[tau % NW][:, 0:WP], MUL
                                     ).then_inc(s_dvef, 1)
            # tail tick: bwd halves first (PE emits MM_b(L-2) before
            # MM_f(L-1) now, so they are ready first), then fwd halves
            tau = L - 1
            vector.wait_ge(s_peb, tau)
            vector.tensor_tensor(ub[tau % NV][:, 0:h], fcol(0, 0, h),
                                 rb[(tau - 1) % NW][:, 0:h], MUL
                                 ).then_inc(s_dveb, 1)
            vector.tensor_tensor(ub[tau % NV][:, h:WP], fcol(0, h, WP),
                                 rb[(tau - 1) % NW][:, h:WP], MUL
                                 ).then_inc(s_dveb, 1)
            vector.wait_ge(s_pef, tau + 1)
            vector.tensor_tensor(vf[tau % NV][:, 0:h], fcol(tau, 0, h),
                                 wf[tau % NW][:, 0:h], MUL
                                 ).then_inc(s_dvef, 1)
            vector.tensor_tensor(vf[tau % NV][:, h:WP], fcol(tau, h, WP),
                                 wf[tau % NW][:, h:WP], MUL
                                 ).then_inc(s_dvef, 1)
    return nc


def _host_constants(fp, tp):
    """g (step-1 fold), mu (mean log growth), c1 (scale) — float64, 8 lanes."""
    alpha0 = np.full(T, INF_MIN)
    alpha0[START] = 0.0
    m0 = tp + alpha0[None, :]
    gmax = m0.max(axis=1, keepdims=True)
    g = gmax[:, 0] + np.log(np.exp(m0 - gmax).sum(axis=1))

    nb = 8
    A64 = np.exp(tp)
    a = fp[:nb, 0, :] + g[None, :]
    vv = np.exp(a - a.max(axis=1, keepdims=True)).T
    ac = a.max(axis=1)
    m_first = float((np.log(vv.sum(axis=0)) + ac).mean())
    for t in range(1, S):
        vv = np.exp(fp[:nb, t, :]).T * (A64 @ vv)
        m = vv.max(axis=0)
        vv /= m[None, :]
        ac += np.log(m)
    m_last = float((np.log(vv.sum(axis=0)) + ac).mean())
    mu = (m_last - m_first) / (S - 1)
    c1 = float(g.max())
    return g, mu, c1


def _layout(batch_len):
    """Greedy lane->core assignment + j-major packed column layout.

    L adapts upward (L=8 default) so that WP fits the PSUM budget
    (2 chains x WP x 4B <= 16KB/partition -> WP <= 2048)."""
    blen = batch_len.astype(np.int64)
    for Lc in (8, 16, 32, 64, 128, 256, 512, 1024):
        ks = S // Lc
        nseg = np.maximum(1, (blen - 2) // Lc + 1)
        nseg = np.where(blen == 1, 1, nseg).astype(np.int64)
        order = np.argsort(-nseg, kind="stable")
        loads = [0] * NCORES
        counts = [0] * NCORES
        core_lanes = [[] for _ in range(NCORES)]
        for lane in order:
            cands = [c for c in range(NCORES) if counts[c] < LANES]
            c = min(cands, key=lambda c: loads[c])
            loads[c] += int(nseg[lane])
            counts[c] += 1
            core_lanes[c].append(int(lane))
        for c in range(NCORES):
            core_lanes[c].sort()
        offs = []          # per core: dict[(lane, j)] -> col
        pc = []
        for c in range(NCORES):
            o = {}
            col = 0
            for j in range(ks):
                for lane in core_lanes[c]:
                    if nseg[lane] > j:
                        o[(lane, j)] = col
                        col += 1
            offs.append(o)
            pc.append(col)
        WP = ((max(pc) + 31) // 32) * 32
        if WP <= 2048:
            return core_lanes, offs, nseg, WP, Lc
    raise AssertionError("no feasible L")


def _prep_inputs(features, batch_len, transitions):
    import ml_dtypes
    bft = ml_dtypes.bfloat16
    f8 = ml_dtypes.float8_e4m3

    perm = np.arange(T)
    perm[SROW], perm[END] = END, SROW
    fp = features[:, :, perm].astype(np.float64)
    tp = transitions[perm][:, perm].astype(np.float64)
    g, mu, c1 = _host_constants(fp, tp)

    A = np.exp(tp - mu)
    A[SROW, :] = 1.0
    A[:, SROW] = 0.0
    A[SROW, SROW] = 1.0
    ewf = np.ascontiguousarray(A.T).astype(bft)   # lhsT fwd: out = A @ v
    ewb = np.ascontiguousarray(A).astype(bft)     # lhsT bwd: out = A.T @ u

    blen = batch_len.astype(np.int64)
    fexp = np.exp(fp).astype(np.float32)
    fexp[:, 0, :] = np.exp(fp[:, 0, :] + g[None, :] - c1)
    dead = np.arange(S)[None, :, None] >= blen[:, None, None]
    fexp = np.where(dead, 0.0, fexp)
    fexp[:, :, SROW] = np.where(dead[:, :, 0], 1.0, 0.0)
    fexp = fexp.astype(bft)
    deadcol = np.zeros((B, 1, T), dtype=bft)
    deadcol[:, 0, SROW] = 1.0
    # matmul step m uses emission col m+1; pad a virtual dead step at m=S-1
    fm = np.concatenate([fexp[:, 1:, :], deadcol], axis=1)  # [B, S, T]

    core_lanes, offs, nseg, WP, Lc = _layout(batch_len)
    ks = S // Lc
    pad_col = np.zeros(T, dtype=bft)
    pad_col[SROW] = 1.0

    # fold v1 into segment 0's first f column so every packed column can
    # start from ones on-device:  f0' = f_{m0} * (A v1) / (A 1)
    Abf = A.astype(bft).astype(np.float64)
    r0 = Abf.sum(axis=1)                                    # A @ 1
    v1all = np.exp(fp[:, 0, :] + g[None, :] - c1)           # [B, T] float64
    Av1 = v1all @ Abf.T                                     # (A @ v1) rows
    f0p = (fm[:, 0, :].astype(np.float64) * Av1 / r0[None, :]).astype(bft)

    in_maps = []
    for cid in range(NCORES):
        ff = np.empty((T, Lc, WP), dtype=f8)
        ff[:] = pad_col.astype(f8)[:, None, None]
        # packed columns: value at block b = fm[lane, j*Lc+b, :]
        lanes_j = [[] for _ in range(ks)]
        for (lane, j), col in offs[cid].items():
            lanes_j[j].append((col, lane))
        for j in range(ks):
            if not lanes_j[j]:
                continue
            cols = np.array([c for c, _ in lanes_j[j]])
            ls = np.array([ln for _, ln in lanes_j[j]])
            ff[:, :, cols] = fm[ls, j * Lc:(j + 1) * Lc, :].transpose(2, 1, 0)
            if j == 0:
                ff[:, 0, cols] = f0p[ls].T
        in_maps.append({"ewf": ewf, "ewb": ewb,
                        "ff": np.ascontiguousarray(ff).reshape(T, Lc * WP)})
    meta = (core_lanes, offs, nseg, WP, Lc)
    return in_maps, A, blen, mu, c1, meta


def _postprocess(res, A, blen, mu, c1, meta):
    core_lanes, offs, nseg, WP, Lc = meta
    out = np.zeros(B, dtype=np.float32)
    for cid in range(NCORES):
        st = np.asarray(res.results[cid]["res"]).astype(np.float64)
        a = st[:, 0:WP]
        u = st[:, WP:2 * WP]
        Aa = A @ a
        o = offs[cid]
        for lane in core_lanes[cid]:
            jm = int(nseg[lane]) - 1
            if jm == 0:
                logs = np.log(a[:, o[(lane, 0)]].sum())
            else:
                dsum = 0.0
                nsum = 0.0
                for j in range(1, jm + 1):
                    dsum += np.log(np.dot(u[:, o[(lane, j)]],
                                          Aa[:, o[(lane, j - 1)]]))
                    if j <= jm - 1:
                        nsum += np.log(a[:, o[(lane, j)]].sum())
                logs = dsum - nsum
            out[lane] = np.float32(
                logs + c1 + (blen[lane] - 1) * mu - 10000.0)
    return out


def run(features, batch_len, transitions, trace=False):
    from concourse.bass_utils import run_bass_kernel_spmd

    features = np.asarray(features, dtype=np.float32)
    batch_len = np.asarray(batch_len, dtype=np.int32)
    transitions = np.asarray(transitions, dtype=np.float32)

    in_maps, A, blen, mu, c1, meta = _prep_inputs(
        features, batch_len, transitions)
    WP, Lc = meta[3], meta[4]
    key = ("nc", WP, Lc)
    if key not in _cache:
        _cache[key] = _build_program(WP, Lc)
    res = None
    for attempt in range(3):
        try:
            res = run_bass_kernel_spmd(_cache[key], in_maps,
                                       list(range(NCORES)), trace=trace)
            break
        except Exception:
            if attempt == 2:
                raise
            import time
            time.sleep(2.0)

    out = _postprocess(res, A, blen, mu, c1, meta)
    if np.isnan(out).any() or np.isinf(out).any():
        res = run_bass_kernel_spmd(_cache[key], in_maps,
                                   list(range(NCORES)), trace=trace)
        out = _postprocess(res, A, blen, mu, c1, meta)
    return out, res


def kernel(features, batch_len, transitions):
    out, _ = run(features, batch_len, transitions, trace=False)
    return out
